# revision 7
# baseline (speedup 1.0000x reference)
import sys

sys.path.insert(0, "/opt/trn_rl_repo")
import numpy as np
import ml_dtypes

import concourse.bass as bass
import concourse.bacc as bacc
import concourse.mybir as mybir
from concourse import tile
from concourse.bass_utils import run_bass_kernel_spmd

# Problem constants (hardcoded per contract)
D_MODEL = 1024
D_STATE = 16
D_CONV = 4
D_INNER = 2048
DT_RANK = 64
B_SZ, L = 2, 2048

NCORES = 8
TPG = 4  # tensor-parallel cores per batch group
ES = D_INNER // TPG  # 512 channels per core
NET = ES // 128  # 4 e-tiles per core
LC = 512  # time chunk
NCH = L // LC  # 4 chunks
DMO = D_MODEL // 128  # 8 d_model tiles

F32 = mybir.dt.float32
BF16 = mybir.dt.bfloat16
F32R = mybir.dt.float32r
AF = mybir.ActivationFunctionType
OP = mybir.AluOpType
RG = [[0, 1, 2, 3], [4, 5, 6, 7]]
BF16_NP = np.dtype(ml_dtypes.bfloat16)


def r32(ap):
    return ap.bitcast(F32R)


def build_nc():
    nc = bacc.Bacc("TRN2", target_bir_lowering=False, debug=False, num_devices=NCORES)

    xhT_d = nc.dram_tensor("xhT", [D_MODEL, L], BF16, kind="ExternalInput")
    xfT_d = nc.dram_tensor("xfT", [D_MODEL, L], BF16, kind="ExternalInput")
    whfT_d = nc.dram_tensor("whfT", [D_MODEL, D_MODEL], BF16, kind="ExternalInput")
    bhf_d = nc.dram_tensor("bhf", [D_MODEL, 1], F32, kind="ExternalInput")
    winxT_d = nc.dram_tensor("winxT", [D_MODEL, ES], BF16, kind="ExternalInput")
    winzT_d = nc.dram_tensor("winzT", [D_MODEL, ES], BF16, kind="ExternalInput")
    convw_d = nc.dram_tensor("convw", [ES, D_CONV], F32, kind="ExternalInput")
    convb_d = nc.dram_tensor("convb", [ES, 1], F32, kind="ExternalInput")
    wxT_d = nc.dram_tensor("wxT", [ES, 96], F32, kind="ExternalInput")
    wdtT_d = nc.dram_tensor("wdtT", [DT_RANK, ES], F32, kind="ExternalInput")
    bdt_d = nc.dram_tensor("bdt", [ES, 1], F32, kind="ExternalInput")
    alog_d = nc.dram_tensor("alog", [ES, D_STATE], F32, kind="ExternalInput")
    dvec_d = nc.dram_tensor("dvec", [ES, 1], F32, kind="ExternalInput")
    woutT_d = nc.dram_tensor("woutT", [ES, D_MODEL], BF16, kind="ExternalInput")
    out_ext = nc.dram_tensor("out", [D_MODEL // TPG, L], F32, kind="ExternalOutput")

    pout_d = nc.dram_tensor("pout", [D_MODEL, L], F32)
    rsout_d = nc.dram_tensor("rsout", [D_MODEL // TPG, L], F32)

    with tile.TileContext(nc) as tc:
        with (
            tc.tile_pool(name="w", bufs=1) as wp,
            tc.tile_pool(name="sb", bufs=2) as sb,
            tc.tile_pool(name="sb3", bufs=3) as sb3,
            tc.tile_pool(name="sb1", bufs=1) as sb1,
            tc.tile_pool(name="scr", bufs=8) as scr,
            tc.tile_pool(name="ps", bufs=3, space="PSUM") as psp,
            tc.tile_pool(name="psb", bufs=2, space="PSUM") as psb,
            tc.tile_pool(name="dram", bufs=2, space="DRAM") as dp,
        ):
            # ---- persistent weights / params ----
            whf_sb = []
            winx_sb = []
            winz_sb = []
            for ki in range(DMO):
                t = wp.tile([128, D_MODEL], BF16, tag=f"whf{ki}")
                nc.sync.dma_start(t[:], whfT_d[ki * 128 : (ki + 1) * 128, :])
                whf_sb.append(t)
                tx = wp.tile([128, ES], BF16, tag=f"winx{ki}")
                nc.sync.dma_start(tx[:], winxT_d[ki * 128 : (ki + 1) * 128, :])
                winx_sb.append(tx)
                tz = wp.tile([128, ES], BF16, tag=f"winz{ki}")
                nc.sync.dma_start(tz[:], winzT_d[ki * 128 : (ki + 1) * 128, :])
                winz_sb.append(tz)
            wx_sb = []
            wout_sb = []
            convw_sb = []
            aneg_sb = []
            for et in range(NET):
                t = wp.tile([128, 96], F32, tag=f"wx{et}")
                nc.sync.dma_start(t[:], wxT_d[et * 128 : (et + 1) * 128, :])
                wx_sb.append(t)
                t = wp.tile([128, D_MODEL], BF16, tag=f"wout{et}")
                nc.sync.dma_start(t[:], woutT_d[et * 128 : (et + 1) * 128, :])
                wout_sb.append(t)
                t = wp.tile([128, D_CONV], F32, tag=f"convw{et}")
                nc.sync.dma_start(t[:], convw_d[et * 128 : (et + 1) * 128, :])
                convw_sb.append(t)
                t = wp.tile([128, D_STATE], F32, tag=f"aneg{et}")
                nc.sync.dma_start(t[:], alog_d[et * 128 : (et + 1) * 128, :])
                nc.scalar.activation(t[:], t[:], AF.Exp)
                nc.vector.tensor_scalar_mul(t[:], t[:], -1.0)
                aneg_sb.append(t)
            wdt_sb = wp.tile([DT_RANK, ES], F32, tag="wdt")
            nc.sync.dma_start(wdt_sb[:], wdtT_d[:, :])
            bhf_sb = wp.tile([128, DMO], F32, tag="bhf")
            for mo in range(DMO):
                nc.sync.dma_start(
                    bhf_sb[:, mo : mo + 1], bhf_d[mo * 128 : (mo + 1) * 128, :]
                )
            convb_sb = wp.tile([128, NET], F32, tag="convb")
            bdt_sb = wp.tile([128, NET], F32, tag="bdt")
            d_sb = wp.tile([128, NET], F32, tag="dvec")
            for et in range(NET):
                nc.sync.dma_start(
                    convb_sb[:, et : et + 1], convb_d[et * 128 : (et + 1) * 128, :]
                )
                nc.sync.dma_start(
                    bdt_sb[:, et : et + 1], bdt_d[et * 128 : (et + 1) * 128, :]
                )
                nc.sync.dma_start(
                    d_sb[:, et : et + 1], dvec_d[et * 128 : (et + 1) * 128, :]
                )
            carry = wp.tile([128, D_STATE * NET], F32, tag="carry")
            nc.gpsimd.memset(carry[:], 0.0)

            # per-chunk state handed from P1 to P2
            xin_prev = [None] * NET
            chunk_state = {}

            def phase1(c):
                t0 = c * LC
                xh_t = sb1.tile([128, DMO, LC], BF16, tag="xh")
                xf_t = sb1.tile([128, DMO, LC], BF16, tag="xf")
                for ki in range(DMO):
                    nc.sync.dma_start(
                        xh_t[:, ki, :], xhT_d[ki * 128 : (ki + 1) * 128, t0 : t0 + LC]
                    )
                    nc.sync.dma_start(
                        xf_t[:, ki, :], xfT_d[ki * 128 : (ki + 1) * 128, t0 : t0 + LC]
                    )
                # hf gate + modulated
                mod_t = sb1.tile([128, DMO, LC], BF16, tag="mod")
                for mo in range(DMO):
                    ps = psp.tile([128, LC], F32, tag="mm")
                    for ki in range(DMO):
                        nc.tensor.matmul(
                            ps[:],
                            whf_sb[ki][:, mo * 128 : (mo + 1) * 128],
                            xh_t[:, ki, :],
                            start=(ki == 0),
                            stop=(ki == DMO - 1),
                        )
                    g = sb3.tile([128, LC], BF16, tag="gate")
                    nc.scalar.activation(
                        g[:], ps[:], AF.Tanh, bias=bhf_sb[:, mo : mo + 1], scale=0.5
                    )
                    nc.vector.scalar_tensor_tensor(
                        mod_t[:, mo, :], g[:], 1.0, xf_t[:, mo, :], OP.add, OP.mult
                    )
                # in_proj x-branch + conv input staging, z-branch
                xin_t = []
                zs_c = []
                xc_c = []
                for et in range(NET):
                    ps = psp.tile([128, LC], F32, tag="mm")
                    for ki in range(DMO):
                        nc.tensor.matmul(
                            ps[:],
                            winx_sb[ki][:, et * 128 : (et + 1) * 128],
                            mod_t[:, ki, :],
                            start=(ki == 0),
                            stop=(ki == DMO - 1),
                        )
                    xt = sb.tile([128, LC + 3], BF16, tag=f"xin{et}")
                    if c == 0:
                        nc.gpsimd.memset(xt[:, 0:3], 0.0)
                    else:
                        nc.scalar.copy(xt[:, 0:3], xin_prev[et][:, LC : LC + 3])
                    nc.scalar.copy(xt[:, 3 : LC + 3], ps[:])
                    xin_t.append(xt)

                    ps2 = psp.tile([128, LC], F32, tag="mm")
                    for ki in range(DMO):
                        nc.tensor.matmul(
                            ps2[:],
                            winz_sb[ki][:, et * 128 : (et + 1) * 128],
                            mod_t[:, ki, :],
                            start=(ki == 0),
                            stop=(ki == DMO - 1),
                        )
                    ztt = sb3.tile([128, LC], F32, tag="ztmp")
                    nc.scalar.activation(ztt[:], ps2[:], AF.Tanh, scale=0.5)
                    zt = sb.tile([128, LC], F32, tag=f"zs{et}")
                    nc.vector.scalar_tensor_tensor(
                        zt[:], ztt[:], 1.0, ps2[:], OP.add, OP.mult
                    )
                    zs_c.append(zt)
                # causal depthwise conv + silu
                for et in range(NET):
                    cacc = sb.tile([128, LC], F32, tag="cacc")
                    nc.vector.tensor_scalar(
                        cacc[:],
                        xin_t[et][:, 0:LC],
                        convw_sb[et][:, 0:1],
                        convb_sb[:, et : et + 1],
                        OP.mult,
                        OP.add,
                    )
                    for k in range(1, D_CONV):
                        nc.vector.scalar_tensor_tensor(
                            cacc[:],
                            xin_t[et][:, k : k + LC],
                            convw_sb[et][:, k : k + 1],
                            cacc[:],
                            OP.mult,
                            OP.add,
                        )
                    ctt = sb3.tile([128, LC], F32, tag="ctmp")
                    nc.scalar.activation(ctt[:], cacc[:], AF.Tanh, scale=0.5)
                    xct = sb.tile([128, LC], F32, tag=f"xc{et}")
                    nc.vector.scalar_tensor_tensor(
                        xct[:], ctt[:], 1.0, cacc[:], OP.add, OP.mult
                    )
                    xc_c.append(xct)
                # x_dbl partial + AllReduce
                ps3 = psp.tile([96, LC], F32, tag="mm")
                for et in range(NET):
                    nc.tensor.matmul(
                        ps3[:],
                        wx_sb[et][:],
                        xc_c[et][:],
                        start=(et == 0),
                        stop=(et == NET - 1),
                    )
                xdbl_st = sb1.tile([96, LC], F32, tag="xdbl_st")
                nc.scalar.copy(xdbl_st[:], ps3[:])
                cin = dp.tile([96, LC], F32, tag="cin")
                nc.sync.dma_start(cin[:], xdbl_st[:])
                cout = dp.tile([96, LC], F32, tag="cout")
                nc.gpsimd.collective_compute(
                    "AllReduce", OP.add, replica_groups=RG, ins=[cin[:]], outs=[cout[:]]
                )
                xdbl = sb.tile([96, LC], F32, tag="xdbl")
                nc.sync.dma_start(xdbl[:], cout[:])
                for et in range(NET):
                    xin_prev[et] = xin_t[et]
                chunk_state[c] = (xdbl, cout, zs_c, xc_c)

            def phase2(c):
                t0 = c * LC
                xdbl, cout, zs_c, xc_c = chunk_state.pop(c)
                dt_c = []
                dtx_c = []
                yac_c = []
                for et in range(NET):
                    ps = psp.tile([128, LC], F32, tag="mm")
                    nc.tensor.matmul(
                        ps[:],
                        wdt_sb[:, et * 128 : (et + 1) * 128],
                        xdbl[0:DT_RANK, :],
                    )
                    dte = sb3.tile([128, LC], F32, tag="dte")
                    nc.scalar.activation(
                        dte[:], ps[:], AF.Exp, bias=bdt_sb[:, et : et + 1]
                    )
                    nc.vector.tensor_scalar_add(dte[:], dte[:], 1.0)
                    dtt = sb1.tile([128, LC], F32, tag=f"dt{et}")
                    nc.scalar.activation(dtt[:], dte[:], AF.Ln)
                    dt_c.append(dtt)
                    dxt = sb1.tile([128, LC], F32, tag=f"dtx{et}")
                    nc.vector.scalar_tensor_tensor(
                        dxt[:], xc_c[et][:], 0.5, dtt[:], OP.mult, OP.mult
                    )
                    dtx_c.append(dxt)
                    yac = sb1.tile([128, LC], F32, tag=f"yac{et}")
                    nc.vector.tensor_scalar_mul(yac[:], xc_c[et][:], d_sb[:, et : et + 1])
                    yac_c.append(yac)
                for s in range(D_STATE):
                    bbc = sb.tile([128, LC], F32, tag="bbc")
                    nc.sync.dma_start(
                        bbc[:], cout[64 + s : 65 + s, :].partition_broadcast(128)
                    )
                    cbc = sb.tile([128, LC], F32, tag="cbc")
                    nc.sync.dma_start(
                        cbc[:], cout[80 + s : 81 + s, :].partition_broadcast(128)
                    )
                    for et in range(NET):
                        col = s * NET + et
                        a_t = scr.tile([128, LC], F32, tag="scr")
                        nc.scalar.activation(
                            a_t[:], dt_c[et][:], AF.Exp, scale=aneg_sb[et][:, s : s + 1]
                        )
                        b_t = scr.tile([128, LC], F32, tag="scr")
                        nc.vector.tensor_mul(b_t[:], dtx_c[et][:], bbc[:])
                        h_t = scr.tile([128, LC], F32, tag="scr")
                        nc.vector.tensor_tensor_scan(
                            h_t[:],
                            a_t[:],
                            b_t[:],
                            carry[:, col : col + 1],
                            OP.mult,
                            OP.add,
                        )
                        nc.scalar.copy(carry[:, col : col + 1], h_t[:, LC - 1 : LC])
                        tmp = scr.tile([128, LC], F32, tag="scr")
                        nc.vector.tensor_mul(tmp[:], h_t[:], cbc[:])
                        nc.vector.tensor_add(yac_c[et][:], yac_c[et][:], tmp[:])
                # gate + out projection partials
                yg_c = []
                for et in range(NET):
                    yg = sb.tile([128, LC], BF16, tag=f"yg{et}")
                    nc.vector.tensor_mul(yg[:], yac_c[et][:], zs_c[et][:])
                    yg_c.append(yg)
                for mo in range(DMO):
                    ps = psp.tile([128, LC], F32, tag="mm")
                    for et in range(NET):
                        nc.tensor.matmul(
                            ps[:],
                            wout_sb[et][:, mo * 128 : (mo + 1) * 128],
                            yg_c[et][:],
                            start=(et == 0),
                            stop=(et == NET - 1),
                        )
                    ost = sb.tile([128, LC], F32, tag="ost")
                    nc.scalar.copy(ost[:], ps[:])
                    nc.sync.dma_start(
                        pout_d[mo * 128 : (mo + 1) * 128, t0 : t0 + LC], ost[:]
                    )

            phase1(0)
            phase1(1)
            phase2(0)
            phase1(2)
            phase2(1)
            phase1(3)
            phase2(2)
            phase2(3)

            nc.gpsimd.collective_compute(
                "ReduceScatter",
                OP.add,
                replica_groups=RG,
                ins=[pout_d[:, :]],
                outs=[rsout_d[:, :]],
            )
            nc.sync.dma_start(out_ext[:, :], rsout_d[:, :])

    nc.compile()
    return nc


_NC_CACHE = None


def get_nc():
    global _NC_CACHE
    if _NC_CACHE is None:
        _NC_CACHE = build_nc()
    return _NC_CACHE


def make_in_maps(x_f, x_h, W_hf, b_hf, W_in, conv_w, conv_b, W_x, W_dt, b_dt, A_log, D, W_out):
    x_f = np.asarray(x_f, np.float32)
    x_h = np.asarray(x_h, np.float32)
    W_hf = np.asarray(W_hf, np.float32)
    b_hf = np.asarray(b_hf, np.float32)
    W_in = np.asarray(W_in, np.float32)
    conv_w = np.asarray(conv_w, np.float32)
    conv_b = np.asarray(conv_b, np.float32)
    W_x = np.asarray(W_x, np.float32)
    W_dt = np.asarray(W_dt, np.float32)
    b_dt = np.asarray(b_dt, np.float32)
    A_log = np.asarray(A_log, np.float32)
    D = np.asarray(D, np.float32)
    W_out = np.asarray(W_out, np.float32)

    whfT = np.ascontiguousarray(W_hf.T).astype(BF16_NP)
    winT = np.ascontiguousarray(W_in.T)  # [1024, 4096]
    wxT = np.ascontiguousarray(W_x.T)  # [2048, 96]
    wdtT = np.ascontiguousarray(W_dt.T)  # [64, 2048]
    woutT = np.ascontiguousarray(W_out.T)  # [2048, 1024]
    in_maps = []
    for core in range(NCORES):
        b, r = core // TPG, core % TPG
        sl = slice(r * ES, (r + 1) * ES)
        zsl = slice(D_INNER + r * ES, D_INNER + (r + 1) * ES)
        in_maps.append(
            {
                "xhT": np.ascontiguousarray(x_h[b].T).astype(BF16_NP),
                "xfT": np.ascontiguousarray(0.5 * x_f[b].T).astype(BF16_NP),
                "whfT": whfT,
                "bhf": 0.5 * b_hf.reshape(D_MODEL, 1),
                "winxT": np.ascontiguousarray(winT[:, sl]).astype(BF16_NP),
                "winzT": np.ascontiguousarray(winT[:, zsl]).astype(BF16_NP),
                "convw": np.ascontiguousarray(conv_w[sl, 0, :]),
                "convb": conv_b[sl].reshape(ES, 1),
                "wxT": np.ascontiguousarray(wxT[sl, :]),
                "wdtT": np.ascontiguousarray(wdtT[:, sl]),
                "bdt": b_dt[sl].reshape(ES, 1),
                "alog": np.ascontiguousarray(A_log[sl, :]),
                "dvec": 0.5 * D[sl].reshape(ES, 1),
                "woutT": np.ascontiguousarray(0.5 * woutT[sl, :]).astype(BF16_NP),
            }
        )
    return in_maps


def assemble(results):
    out = np.empty((B_SZ, L, D_MODEL), np.float32)
    for b in range(B_SZ):
        shards = [results[b * TPG + r]["out"] for r in range(TPG)]
        outT = np.concatenate(shards, axis=0)  # [1024, 2048]
        out[b] = outT.T
    return out


def kernel_ext(inputs, trace=False):
    nc = get_nc()
    in_maps = make_in_maps(**inputs)
    res = run_bass_kernel_spmd(nc, in_maps, list(range(NCORES)), trace=trace)
    return assemble(res.results), res


def kernel(**inputs):
    out, _ = kernel_ext(inputs)
    return out
